# revision 1
# baseline (speedup 1.0000x reference)
"""MixedScoreMultiHeadAttention Trainium2 kernel (PE-centric pipeline).

Data-parallel over batch: 32 batches -> 8 cores x 4 batches.

Per (b):
  dot_h = q_h k_h^T  (per head, PE)  -> flattened r-major into rhs rows
  layer1: T[(h,m), pts] = a[h,m]*dot_h[pts] + c[h,m]*Y[pts]   (PE matmul,
          constant stationary [17,128]; bias b[h,m] folded into relu evac)
  R = relu(T + b)                     (ACT/DVE evacuation from PSUM, fp16)
  layer2: mixed^T[pts, h] via stationary-swapped matmul (lhsT = R data,
          rhs = block-diag w2 [128,8]) -> PSUM [c, (r-grp, h)] full-partition
  exp-evac (ACT Exp) -> w_sb [c, (r,h)] fp32
  AV: out[r, 17] = w^T-slice.T @ [v|1]  (ones col gives softmax denominator)
  normalize by reciprocal of col 16 -> out rows

mix2 bias b2 is dropped (constant shift is softmax-invariant); 1/sqrt(D) is
folded into Wq host-side.
"""
import sys

sys.path.insert(0, "/opt/trn_rl_repo")

import numpy as np
from contextlib import ExitStack

import concourse.bass as bass
import concourse.mybir as mybir
import concourse.tile as tile
from concourse import bacc
from concourse.bass_utils import run_bass_kernel_spmd
from concourse.masks import make_identity

B, R, C, E, H, D, MS = 32, 128, 128, 256, 16, 16, 16
NCORES = 8
BL = B // NCORES  # batches per core: 4
TOK = BL * R      # 512 tokens per core per side
PTS = R * C       # 16384 score points per (b)

FP32 = mybir.dt.float32
FP16 = mybir.dt.float16
AF = mybir.ActivationFunctionType
ALU = mybir.AluOpType



def build_kernel():
    nc = bacc.Bacc("TRN2", target_bir_lowering=False, debug=False,
                   num_devices=NCORES)

    x_r = nc.dram_tensor("x_r", [TOK, E], FP32, kind="ExternalInput").ap()
    x_c = nc.dram_tensor("x_c", [TOK, E], FP32, kind="ExternalInput").ap()
    cost = nc.dram_tensor("cost", [BL, R, C], FP32, kind="ExternalInput").ap()
    # Wq pre-scaled by 1/sqrt(D) host-side; head-padding to 32-col slots
    # (for 32-aligned projection PSUM rows) happens on-chip.
    wq_d = nc.dram_tensor("Wq", [E, E], FP32, kind="ExternalInput").ap()
    wk_d = nc.dram_tensor("Wk", [E, E], FP32, kind="ExternalInput").ap()
    wv_d = nc.dram_tensor("Wv", [E, E], FP32, kind="ExternalInput").ap()
    # layer1 stationary [17, 256]: col (half*128 + (h%8)*16 + m):
    #   row h' = a[h,m] iff h'==h; row 16 = c[h,m]
    w1_d = nc.dram_tensor("W1L", [17, 2 * 128], FP32,
                          kind="ExternalInput").ap()
    # layer2 moving [128, 16]: col (half*8 + j): row hm = w2[half*8+j, m]
    # iff hm == ((j)*16+m) else 0
    w2_d = nc.dram_tensor("W2L", [128, 16], FP32, kind="ExternalInput").ap()
    # relu bias per (h,m) row: bcol2[hm, half] = b1[half*8 + hm//16, hm%16]
    bc_d = nc.dram_tensor("bcol2", [128, 2], FP32, kind="ExternalInput").ap()
    out_d = nc.dram_tensor("out", [BL, R, H * D], FP32,
                           kind="ExternalOutput").ap()

    with tile.TileContext(nc) as tc, ExitStack() as ctx:
        const_p = ctx.enter_context(tc.tile_pool(name="const", bufs=1))
        inx_p = ctx.enter_context(tc.tile_pool(name="inx", bufs=2))
        w_p = ctx.enter_context(tc.tile_pool(name="wts", bufs=1))
        xt_p = ctx.enter_context(tc.tile_pool(name="xt", bufs=1))
        qkv_p = ctx.enter_context(tc.tile_pool(name="qkv", bufs=1))
        x4_p = ctx.enter_context(tc.tile_pool(name="x4", bufs=1))
        rhs_p = ctx.enter_context(tc.tile_pool(name="rhs", bufs=2))
        rr_p = ctx.enter_context(tc.tile_pool(name="rr", bufs=6))
        wsb_p = ctx.enter_context(tc.tile_pool(name="wsb", bufs=2))
        fout_p = ctx.enter_context(tc.tile_pool(name="fout", bufs=1))
        small_p = ctx.enter_context(tc.tile_pool(name="small", bufs=4))
        ps_tr = ctx.enter_context(
            tc.tile_pool(name="pstr", bufs=1, space="PSUM"))
        ps_big = ctx.enter_context(
            tc.tile_pool(name="psb", bufs=4, space="PSUM"))
        ps_l2 = ctx.enter_context(
            tc.tile_pool(name="psl2", bufs=2, space="PSUM"))
        ps_av = ctx.enter_context(
            tc.tile_pool(name="psa", bufs=1, space="PSUM"))

        ident = const_p.tile([128, 128], FP32)
        make_identity(nc, ident[:])

        # ---- small weight/const loads
        w1f = inx_p.tile([17, 2 * 128], FP32, tag="w1f")
        nc.sync.dma_start(w1f[:], w1_d[:])
        w1l = const_p.tile([17, 2 * 128], FP16)
        nc.vector.tensor_copy(w1l[:], w1f[:])

        w2f = inx_p.tile([128, 16], FP32, tag="w2f")
        nc.sync.dma_start(w2f[:], w2_d[:])
        w2l = const_p.tile([128, 16], FP16)
        nc.vector.tensor_copy(w2l[:], w2f[:])

        bcol2 = const_p.tile([128, 2], FP32)
        nc.sync.dma_start(bcol2[:], bc_d[:])

        # ---- QKV weights fp16 (q/k padded on-chip: head h -> 32-col slot)
        wt16 = {}
        for name, dram in (("q", wq_d), ("k", wk_d), ("v", wv_d)):
            halves = []
            for eh in range(2):
                w32 = inx_p.tile([128, E], FP32, tag="wload")
                nc.sync.dma_start(w32[:], dram[eh * 128:(eh + 1) * 128, :])
                ncols = E if name == "v" else 2 * E
                w16 = w_p.tile([128, ncols], FP16, tag=f"w16{name}{eh}",
                               name=f"w16{name}{eh}")
                if name == "v":
                    nc.vector.tensor_copy(w16[:], w32[:])
                else:
                    nc.gpsimd.memset(w16[:], 0.0)
                    w16v = w16[:].rearrange("p (h x) -> p h x", h=H)
                    w32v = w32[:].rearrange("p (h x) -> p h x", h=H)
                    nc.vector.tensor_copy(w16v[:, :, 0:D], w32v[:])
                halves.append(w16)
            wt16[name] = halves

        # ---- x load + PE transpose -> xT fp16 [2 e-halves][128, TOK]
        xT = {}
        for name, dram in (("r", x_r), ("c", x_c)):
            xt0 = xt_p.tile([128, TOK], FP16, tag=f"xT{name}0")
            xt1 = xt_p.tile([128, TOK], FP16, tag=f"xT{name}1")
            xT[name] = [xt0, xt1]
            for t in range(BL):
                x32 = inx_p.tile([128, E], FP32, tag="xload")
                nc.sync.dma_start(x32[:], dram[t * 128:(t + 1) * 128, :])
                for eh in range(2):
                    pst = ps_tr.tile([128, 128], FP32, tag="pstr")
                    nc.tensor.transpose(
                        pst[:], x32[:, eh * 128:(eh + 1) * 128], ident[:])
                    nc.vector.tensor_copy(
                        xT[name][eh][:, t * 128:(t + 1) * 128], pst[:])

        # ---- cost -> fp16 [r, c] tiles (r-major flatten later)
        y16 = []
        for b in range(BL):
            c32 = inx_p.tile([128, C], FP32, tag="cload")
            nc.sync.dma_start(c32[:], cost[b])
            y1 = const_p.tile([128, C], FP16, name=f"y16_{b}", tag=f"y16_{b}")
            nc.vector.tensor_copy(y1[:], c32[:])
            y16.append(y1)

        # ---- projections: qT/kT per-head tiles [16, TOK] fp16
        qT, kT = [], []
        for proj, dst in (("q", qT), ("k", kT)):
            for mh in range(4):  # head-quad tiles (4 heads x 32 rows)
                ps = ps_big.tile([128, TOK], FP32, tag="psbig")
                for eh in range(2):
                    nc.tensor.matmul(
                        ps[:],
                        wt16[proj][eh][:, mh * 128:(mh + 1) * 128],
                        xT["r" if proj == "q" else "c"][eh][:],
                        start=(eh == 0), stop=(eh == 1))
                # pack 3 head evacs in one 96-row op (PE matmul operands may
                # sit at base partition 0/32/64; 96 is invalid -> separate)
                quad = qkv_p.tile([96, TOK], FP16, tag=f"{proj}Q{mh}",
                                  name=f"{proj}Q{mh}")
                last = qkv_p.tile([16, TOK], FP16, tag=f"{proj}L{mh}",
                                  name=f"{proj}L{mh}")
                if mh % 2 == 0:
                    nc.scalar.copy(quad[:], ps[0:96, :])
                    nc.vector.tensor_copy(last[:], ps[96:112, :])
                else:
                    nc.vector.tensor_copy(quad[:], ps[0:96, :])
                    nc.scalar.copy(last[:], ps[96:112, :])
                for hh in range(4):
                    dst.append(quad[hh * 32:hh * 32 + 16, :] if hh < 3
                               else last[:])

        # ---- v natural [c, hd] fp32 interleaved with ones col -> vhat
        vhat = []
        for b in range(BL):
            vh = qkv_p.tile([128, 17 * H], FP32, tag=f"vhat{b}",
                            name=f"vhat{b}")
            vh3 = vh[:].rearrange("p (h x) -> p h x", h=H)
            nc.gpsimd.memset(vh3[:, :, 16:17], 1.0)
            ps = ps_big.tile([128, E], FP32, tag="psbig")
            for eh in range(2):
                nc.tensor.matmul(
                    ps[:], xT["c"][eh][:, b * 128:(b + 1) * 128],
                    wt16["v"][eh][:], start=(eh == 0), stop=(eh == 1))
            nc.scalar.copy(
                vh3[:, :, 0:16], ps[:].rearrange("p (h x) -> p h x", h=H))
            vhat.append(vh)

        # ---- dots: X4[h] fp16 [r, (b, c)]
        x4s = []
        for h in range(H):
            x4 = x4_p.tile([128, BL * C], FP16, tag=f"x4_{h}",
                           name=f"x4_{h}")
            psd = ps_big.tile([128, BL * C], FP32, tag="psbig")
            for b in range(BL):
                nc.tensor.matmul(
                    psd[:, b * 128:(b + 1) * 128],
                    qT[h][:, b * 128:(b + 1) * 128],
                    kT[h][:, b * 128:(b + 1) * 128])
            if h % 2 == 0:
                nc.scalar.copy(x4[:], psd[:])
            else:
                nc.vector.tensor_copy(x4[:], psd[:])
            x4s.append(x4)

        # ---- per (b): layer1+relu, layer2, exp, AV
        fouts = [fout_p.tile([128, H * D], FP32, tag=f"fo{b}", name=f"fo{b}")
                 for b in range(BL)]
        for b in range(BL):
            rhs = rhs_p.tile([17, PTS], FP16, tag="rhs")
            for h in range(H):
                nc.sync.dma_start(rhs[h:h + 1, :],
                                  x4s[h][:, b * 128:(b + 1) * 128])
            nc.sync.dma_start(rhs[16:17, :], y16[b][:])

            for half in range(2):
                wsb = wsb_p.tile([128, 8 * C], FP32)
                for grp in range(2):  # 64 r's per group
                    ps2 = ps_l2.tile([128, 512], FP32)
                    for cki in range(16):  # layer1 chunks of 512 pts
                        ck = grp * 16 + cki
                        rr = rr_p.tile([128, 512], FP16, tag="rr")
                        ps1 = ps_big.tile([128, 512], FP32, tag="psbig")
                        nc.tensor.matmul(
                            ps1[:], w1l[:, half * 128:(half + 1) * 128],
                            rhs[:, ck * 512:(ck + 1) * 512])
                        if ck % 2 == 0:
                            nc.scalar.activation(
                                rr[:], ps1[:], AF.Relu,
                                bias=bcol2[:, half:half + 1])
                        else:
                            nc.vector.tensor_scalar(
                                rr[:], ps1[:], bcol2[:, half:half + 1],
                                0.0, ALU.add, ALU.max)
                        for s in range(4):  # layer2 per 128-pt subchunk
                            rloc = cki * 4 + s
                            nc.tensor.matmul(
                                ps2[:, rloc * 8:rloc * 8 + 8],
                                rr[:, s * 128:(s + 1) * 128],
                                w2l[:, half * 8:(half + 1) * 8])
                    nc.scalar.activation(
                        wsb[:, grp * 512:(grp + 1) * 512], ps2[:], AF.Exp)

                # AV + normalize for the 8 heads of this half
                psa = ps_av.tile([128, 17 * 8], FP32)
                wsb4 = wsb[:].rearrange("p (g s h) -> p g s h", g=2, s=64)
                for hl in range(8):
                    h = half * 8 + hl
                    nc.tensor.matmul(
                        psa[:, hl * 17:(hl + 1) * 17],
                        wsb4[:, :, :, hl],
                        vhat[b][:, h * 17:(h + 1) * 17])
                rec = small_p.tile([128, 8], FP32, tag="rec")
                psa3 = psa[:].rearrange("p (x y) -> p x y", x=8)
                nc.vector.reciprocal(rec[:], psa3[:, :, 16])
                for hl in range(8):
                    h = half * 8 + hl
                    nc.vector.tensor_scalar(
                        fouts[b][:, h * D:(h + 1) * D], psa3[:, hl, 0:16],
                        rec[:, hl:hl + 1], None, ALU.mult)

        for b in range(BL):
            nc.sync.dma_start(out_d[b], fouts[b][:])

    nc.compile()
    return nc


_cache = {}


def kernel(**inputs):
    row_emb = np.asarray(inputs["row_emb"], dtype=np.float32)
    col_emb = np.asarray(inputs["col_emb"], dtype=np.float32)
    cost_mat = np.asarray(inputs["cost_mat"], dtype=np.float32)
    Wq = np.asarray(inputs["Wq"], dtype=np.float32)
    Wk = np.asarray(inputs["Wk"], dtype=np.float32)
    Wv = np.asarray(inputs["Wv"], dtype=np.float32)
    m1w = np.asarray(inputs["mix1_weight"], dtype=np.float32)
    m1b = np.asarray(inputs["mix1_bias"], dtype=np.float32)
    m2w = np.asarray(inputs["mix2_weight"], dtype=np.float32)

    a1 = m1w[:, 0, :]
    c1 = m1w[:, 1, :]
    w2 = m2w[:, :, 0]

    if "nc" not in _cache:
        _cache["nc"] = build_kernel()
    nc = _cache["nc"]

    wq_s = Wq * (1.0 / np.sqrt(D))
    wk_p = Wk

    w1l = np.zeros((17, 256), dtype=np.float32)
    w2l = np.zeros((128, 16), dtype=np.float32)
    bcol2 = np.zeros((128, 2), dtype=np.float32)
    for h in range(H):
        half, hl = h // 8, h % 8
        for m in range(MS):
            col = half * 128 + hl * 16 + m
            w1l[h, col] = a1[h, m]
            w1l[16, col] = c1[h, m]
            w2l[hl * 16 + m, half * 8 + hl] = w2[h, m]
            bcol2[hl * 16 + m, half] = m1b[h, m]

    in_maps = []
    for i in range(NCORES):
        sl = slice(i * BL, (i + 1) * BL)
        in_maps.append({
            "x_r": row_emb[sl].reshape(TOK, E),
            "x_c": col_emb[sl].reshape(TOK, E),
            "cost": cost_mat[sl],
            "Wq": wq_s, "Wk": wk_p, "Wv": Wv,
            "W1L": w1l, "W2L": w2l, "bcol2": bcol2,
        })
    res = run_bass_kernel_spmd(nc, in_maps, list(range(NCORES)))
    out = np.concatenate([res.results[i]["out"] for i in range(NCORES)],
                         axis=0)
    return out.astype(np.float32)



# revision 2
# speedup vs baseline: 1.0464x; 1.0464x over previous
"""MixedScoreMultiHeadAttention TRN2 kernel, v2.

Data-parallel over batch: 32 batches -> 8 cores x 4 batches (BL=4).

Host-side prep (free): x transposed to [E, TOK] fp16, Wq/Wk padded to
32-col head slots and packed fp16, cost flattened r-major fp16, layer1
stationary [17, 256], layer2 block-diag sign/weight matrix, bias cols.

Device pipeline per core:
  proj q/k  -> quad tiles [128, TOK] fp16 (4 head-slots x 32 rows)
  dots      -> x4all [r, (h, b, c)] fp16 (PE, per head-pair psum)
  vhat      -> [c, (h, d|1)] fp16 with ones column for softmax denom
  per b: rhs assembly (2 DMAs) -> [17, 16384] fp16 (16 dot rows + cost)
  per (b, half): 16x { 2 layer1 mm [17,128]x[17,1024] -> psum [128,1024],
                       relu+bias evac (DVE/ACT balanced) -> rr fp16,
                       8 layer2 mm rr[:,128k]x w2l -> ps2[c,(r,h)] }
                 2x exp evac [128,512] -> wsb fp16
                 16 AV mm wsb[c,r-strided] x vhat -> psa [r, (h,17)]
                 reciprocal + per-head normalize -> fout
  out DMA per b.
"""
import sys

sys.path.insert(0, "/opt/trn_rl_repo")

import numpy as np
from contextlib import ExitStack

import concourse.bass as bass
import concourse.mybir as mybir
import concourse.tile as tile
from concourse import bacc
from concourse.bass_utils import run_bass_kernel_spmd

B, R, C, E, H, D, MS = 32, 128, 128, 256, 16, 16, 16
NCORES = 8
BL = B // NCORES   # 4 batches per core
TOK = BL * R       # 512 tokens per core per side
PTS = R * C        # 16384 points per batch

FP32 = mybir.dt.float32
FP16 = mybir.dt.float16
AF = mybir.ActivationFunctionType
ALU = mybir.AluOpType


class Balancer:
    """Greedy DVE/ACT load balancing for PSUM-exit elementwise ops."""

    def __init__(self, nc):
        self.nc = nc
        self.load = {"D": 0.0, "A": 0.0}

    @staticmethod
    def _cost(eng, n, psum_src):
        if eng == "D":
            return n * 1.0417 + (125.0 if psum_src else 60.0)
        return n * 0.8333 + 185.0

    def _pick(self, n, psum_src, forbid=()):
        e = min((x for x in "DA" if x not in forbid),
                key=lambda x: self.load[x])
        self.load[e] += self._cost(e, n, psum_src)
        return e

    def charge(self, eng, n, psum_src=True):
        self.load[eng] += self._cost(eng, n, psum_src)

    def copy(self, out, in_, n):
        if self._pick(n, True) == "D":
            self.nc.vector.tensor_copy(out, in_)
        else:
            self.nc.scalar.copy(out, in_)

    def relu(self, out, in_, bias_ap, n, forbid=()):
        if self._pick(n, True, forbid) == "D":
            self.nc.vector.tensor_scalar(out, in_, bias_ap, 0.0,
                                         ALU.add, ALU.max)
        else:
            self.nc.scalar.activation(out, in_, AF.Relu, bias=bias_ap)


def build_kernel():
    nc = bacc.Bacc("TRN2", target_bir_lowering=False, debug=False,
                   num_devices=NCORES)

    xr_d = nc.dram_tensor("xrT", [128, 2 * TOK], FP16,
                          kind="ExternalInput").ap()
    xc_d = nc.dram_tensor("xcT", [128, 2 * TOK], FP16,
                          kind="ExternalInput").ap()
    wqk_d = nc.dram_tensor("wqk", [128, 2 * 2 * 768], FP16,
                           kind="ExternalInput").ap()
    wv_d = nc.dram_tensor("wv", [128, 2 * E], FP16,
                          kind="ExternalInput").ap()
    w1_d = nc.dram_tensor("w1l", [17, 256], FP16, kind="ExternalInput").ap()
    w2_d = nc.dram_tensor("w2l", [128, 16], FP16, kind="ExternalInput").ap()
    bc_d = nc.dram_tensor("bcol2", [128, 2], FP32, kind="ExternalInput").ap()
    cost_d = nc.dram_tensor("cost16", [BL, PTS], FP16,
                            kind="ExternalInput").ap()
    out_d = nc.dram_tensor("out", [BL, R, H * D], FP32,
                           kind="ExternalOutput").ap()
    # DRAM bounce for dot-score transposition, one tensor per head-quad
    # per batch-pair (b0/b1 critical-path, b2/b3 deferred)
    scr01s = [nc.dram_tensor(f"scr01_{q}", [128, 4 * 2 * C], FP16,
                             kind="Internal").ap() for q in range(4)]
    scr23s = [nc.dram_tensor(f"scr23_{q}", [128, 4 * 2 * C], FP16,
                             kind="Internal").ap() for q in range(4)]

    with tile.TileContext(nc) as tc, ExitStack() as ctx:
        const_p = ctx.enter_context(tc.tile_pool(name="const", bufs=1))
        big_p = ctx.enter_context(tc.tile_pool(name="big", bufs=1))
        rhs_p = ctx.enter_context(tc.tile_pool(name="rhs", bufs=2))
        rr_p = ctx.enter_context(tc.tile_pool(name="rr", bufs=8))
        wsb_p = ctx.enter_context(tc.tile_pool(name="wsb", bufs=4))
        fout_p = ctx.enter_context(tc.tile_pool(name="fout", bufs=1))
        small_p = ctx.enter_context(tc.tile_pool(name="small", bufs=4))
        psA = ctx.enter_context(
            tc.tile_pool(name="psA", bufs=3, space="PSUM"))   # [128,1024] x3
        ps2_p = ctx.enter_context(
            tc.tile_pool(name="ps2", bufs=2, space="PSUM"))   # [128,512] x2

        bal = Balancer(nc)

        # ---- PE warmup: ~4us of dummy matmuls on a zeroed tile so the
        # p-state ramp (2-4x slower first 3us) completes before real work
        zwarm = const_p.tile([128, 512], FP16)
        nc.gpsimd.memset(zwarm[:], 0.0)
        wps = ps2_p.tile([128, 512], FP32, tag="ps2", name="wps")
        for i in range(7):
            nc.tensor.matmul(wps[:], zwarm[:, 0:128], zwarm[:],
                             start=(i == 0), stop=(i == 6))

        # ---- const loads (single DMAs, host-prepped layouts)
        xr2 = const_p.tile([128, 2 * TOK], FP16)   # [p, (eh, tok)]
        xc2 = const_p.tile([128, 2 * TOK], FP16)
        wqk16 = const_p.tile([128, 2 * 2 * 768], FP16)  # [p, (eh, q768|k768)]
        wv16 = const_p.tile([128, 2 * E], FP16)         # [p, (eh, 256)]
        # load order: q-projection deps first so PE starts ASAP
        nc.sync.dma_start(wqk16[:, 0:1536], wqk_d[:, 0:1536])
        nc.sync.dma_start(xr2[:], xr_d[:])
        nc.sync.dma_start(wqk16[:, 1536:3072], wqk_d[:, 1536:3072])
        nc.sync.dma_start(xc2[:], xc_d[:])
        w1sb = const_p.tile([17, 256], FP16)
        nc.gpsimd.dma_start(w1sb[:], w1_d[:])
        w2sb = const_p.tile([128, 16], FP16)
        nc.gpsimd.dma_start(w2sb[:], w2_d[:])
        bc2 = const_p.tile([128, 2], FP32)
        nc.gpsimd.dma_start(bc2[:], bc_d[:])
        nc.gpsimd.dma_start(wv16[:], wv_d[:])

        # ---- prep, mh-major: proj(q,k) for mh then dots for its 4 heads,
        #      so scratch DMAs start as early as possible
        quads = {}
        x4all = big_p.tile([128, H * BL * C], FP16, name="x4all")

        def headT(proj, h):
            q6, sl = h // 3, h % 3
            t = quads[(proj, q6 // 2)]
            co = (q6 % 2) * TOK
            return t[sl * 32:sl * 32 + 16, co:co + TOK]

        # rhs tiles for b0/b1 up front; cost rows load immediately, dot
        # rows arrive as quad-sized stage2 DMAs pipelined into the mh loop
        # views [r, h4, b2, c] for the stage2 per-batch slices
        s01v = [sd[:].rearrange("r (h b c) -> r h b c", h=4, b=2)
                for sd in scr01s]
        s23v = [sd[:].rearrange("r (h b c) -> r h b c", h=4, b=2)
                for sd in scr23s]
        rhs0 = rhs_p.tile([17, PTS], FP16, tag="rhs", name="rhs0")
        rhs1 = rhs_p.tile([17, PTS], FP16, tag="rhs", name="rhs1")
        nc.scalar.dma_start(rhs0[16:17, :], cost_d[0:1, :])
        nc.scalar.dma_start(rhs1[16:17, :], cost_d[1:2, :])

        def emit_proj(pt):
            # pairtile pt covers 6-quads 2pt, 2pt+1 (3 heads each)
            for proj, qofs in (("q", 0), ("k", 768)):
                ps = psA.tile([128, 1024], FP32, tag="psA", name="ps")
                for qp in range(2):
                    q6 = pt * 2 + qp
                    for eh in range(2):
                        nc.tensor.matmul(
                            ps[0:96, qp * TOK:(qp + 1) * TOK],
                            wqk16[:, eh * 1536 + qofs + q6 * 128:
                                  eh * 1536 + qofs + q6 * 128 + 96],
                            xr2[:, eh * TOK:(eh + 1) * TOK] if proj == "q"
                            else xc2[:, eh * TOK:(eh + 1) * TOK],
                            start=(eh == 0), stop=(eh == 1))
                qt = big_p.tile([96, 2 * TOK], FP16, tag=f"{proj}T{pt}",
                                name=f"{proj}T{pt}")
                bal.copy(qt[:], ps[0:96, :], 1024)
                quads[(proj, pt)] = qt

        vhats = {}
        vh_tiles = {}
        for b in range(BL):
            vh = big_p.tile([128, 17 * H], FP16, tag=f"vhat{b}",
                            name=f"vhat{b}")
            vh_tiles[b] = vh
            nc.gpsimd.memset(
                vh[:].rearrange("p (h x) -> p h x", h=H)[:, :, 16:17], 1.0)

        def emit_vhat(b):
            vh = vh_tiles[b]
            vh3 = vh[:].rearrange("p (h x) -> p h x", h=H)
            ps = psA.tile([128, 1024], FP32, tag="psA", name="ps")
            for eh in range(2):
                nc.tensor.matmul(
                    ps[:, 0:E],
                    xc2[:, eh * TOK + b * 128:eh * TOK + (b + 1) * 128],
                    wv16[:, eh * E:(eh + 1) * E],
                    start=(eh == 0), stop=(eh == 1))
            bal.copy(vh3[:, :, 0:16],
                     ps[:, 0:E].rearrange("p (h x) -> p h x", h=H), E)
            vhats[b] = vh

        # proj runs 2 mh ahead of dots; vhat interleaved to fill gaps
        emit_proj(0)
        emit_proj(1)
        for mh in range(4):
            for hp2 in range(2):   # dots for heads 4mh .. 4mh+3
                ps = psA.tile([128, 1024], FP32, tag="psA", name="ps")
                for hh in range(2):
                    h = mh * 4 + hp2 * 2 + hh
                    for b in range(BL):
                        nc.tensor.matmul(
                            ps[:, hh * 512 + b * 128:hh * 512 + (b + 1) * 128],
                            headT("q", h)[:, b * 128:(b + 1) * 128],
                            headT("k", h)[:, b * 128:(b + 1) * 128])
                hbase = mh * 4 + hp2 * 2
                bal.copy(x4all[:, hbase * 512:(hbase + 2) * 512], ps[:], 1024)
            # stage1 for b0/b1 only (first 256 cols of each h-block)
            nc.sync.dma_start(
                scr01s[mh][:],
                x4all[:].rearrange("r (h x) -> r h x", h=H)
                [:, mh * 4:(mh + 1) * 4, 0:256])
            # stage2 quad DMAs: b0 on SP behind its stage1, b1 on Pool
            for eng, bb, rhsq in ((nc.sync, 0, rhs0), (nc.gpsimd, 1, rhs1)):
                eng.dma_start(
                    rhsq[mh * 4:(mh + 1) * 4, :].rearrange(
                        "h (r c) -> h r c", c=C),
                    s01v[mh][:, :, bb, :].rearrange("r h c -> h r c"))
            if mh == 1:
                emit_proj(2)

        for b in range(BL):
            emit_vhat(b)

        # ---- per-batch MLP + softmax + AV
        fouts = [fout_p.tile([128, H * D], FP32, tag=f"fo{b}", name=f"fo{b}")
                 for b in range(BL)]

        # deferred stage1 for b2/b3 (off the critical path)
        for q in range(4):
            nc.sync.dma_start(
                scr23s[q][:],
                x4all[:].rearrange("r (h x) -> r h x", h=H)
                [:, q * 4:(q + 1) * 4, 256:512])

        def assemble(b):
            rhs = rhs_p.tile([17, PTS], FP16, tag="rhs", name="rhs")
            # dots rows: dst (h; r, c) <- scratch [h, r, b, c] slice
            for q in range(4):
                nc.sync.dma_start(
                    rhs[q * 4:(q + 1) * 4, :].rearrange(
                        "h (r c) -> h r c", c=C),
                    s23v[q][:, :, b - 2, :].rearrange("r h c -> h r c"))
            nc.sync.dma_start(rhs[16:17, :], cost_d[b:b + 1, :])
            return rhs

        def make_tail(b, half, wsbs):
            # AV + normalize for (b, half), deferred into the next half's
            # pair loop so the PE never stalls on the ACT-queued exp
            def tail():
                psa = ps2_p.tile([128, 17 * 8], FP32, tag="ps2", name="psa")
                for grp in range(2):
                    w4 = wsbs[grp][:].rearrange("p (r h) -> p r h", h=8)
                    for hl in range(8):
                        h = half * 8 + hl
                        nc.tensor.matmul(
                            psa[grp * 64:(grp + 1) * 64,
                                hl * 17:hl * 17 + 17],
                            w4[:, :, hl],
                            vhats[b][:, h * 17:(h + 1) * 17])
                psa3 = psa[:].rearrange("p (x y) -> p x y", x=8)
                rec = small_p.tile([128, 8], FP32, tag="rec")
                nc.vector.reciprocal(rec[:], psa3[:, :, 16])
                bal.charge("D", 8)
                recb = rec[:].rearrange(
                    "p (h o) -> p h o", o=1).broadcast_to([128, 8, 16])
                nc.vector.tensor_tensor(
                    fouts[b][:, half * 128:(half + 1) * 128].rearrange(
                        "p (h x) -> p h x", h=8),
                    psa3[:, :, 0:16], recb, ALU.mult)
                bal.charge("D", 128)
                if half == 1:
                    nc.sync.dma_start(out_d[b], fouts[b][:])
            return tail

        rhss = {0: rhs0, 1: rhs1}
        prev_tail = None
        for b in range(BL):
            if b + 2 < BL:
                rhss[b + 2] = assemble(b + 2)
            rhs = rhss.pop(b)
            for half in range(2):
                w1h = w1sb[:, half * 128:(half + 1) * 128]
                w2h = w2sb[:, half * 8:(half + 1) * 8]
                bch = bc2[:, half:half + 1]
                ps2s = []
                pend = []   # (rr, pair) with layer2 not yet emitted

                def emit_l2(rr, pair, ps2s=ps2s, w2h=w2h):
                    if pair % 8 == 0:
                        ps2t = ps2_p.tile([128, 512], FP32, tag="ps2",
                                          name="ps2t")
                        ps2s.append(ps2t)
                    for s in range(8):
                        rloc = (pair % 8) * 8 + s
                        nc.tensor.matmul(
                            ps2s[-1][:, rloc * 8:rloc * 8 + 8],
                            rr[:, s * 128:(s + 1) * 128], w2h)

                wsbs = []
                for pair in range(16):
                    ps1 = psA.tile([128, 1024], FP32, tag="psA")
                    for k in range(2):
                        ck = pair * 2 + k
                        nc.tensor.matmul(
                            ps1[:, k * 512:(k + 1) * 512], w1h,
                            rhs[:, ck * 512:(ck + 1) * 512])
                    rr = rr_p.tile([128, 1024], FP16, tag="rr")
                    bal.relu(rr[:], ps1[:], bch, 1024)
                    pend.append((rr, pair))
                    if len(pend) > 3:
                        emit_l2(*pend.pop(0))
                    if pair == 4 and prev_tail is not None:
                        prev_tail()
                        prev_tail = None
                    # grp0 fully written once l2(p7) emitted -> exp it early
                    if pend and pend[0][1] == 8 and len(ps2s) == 1:
                        wsb = wsb_p.tile([128, 512], FP16, tag="wsb")
                        nc.scalar.activation(wsb[:], ps2s[0][:], AF.Exp)
                        bal.charge("A", 512)
                        wsbs.append(wsb)
                while pend:
                    emit_l2(*pend.pop(0))
                wsb = wsb_p.tile([128, 512], FP16, tag="wsb")
                nc.scalar.activation(wsb[:], ps2s[1][:], AF.Exp)
                bal.charge("A", 512)
                wsbs.append(wsb)
                prev_tail = make_tail(b, half, wsbs)

        prev_tail()

    nc.compile()
    return nc


_cache = {}


def _prep(inputs):
    row_emb = np.asarray(inputs["row_emb"], dtype=np.float32)
    col_emb = np.asarray(inputs["col_emb"], dtype=np.float32)
    cost_mat = np.asarray(inputs["cost_mat"], dtype=np.float32)
    Wq = np.asarray(inputs["Wq"], dtype=np.float32) / np.sqrt(D)
    Wk = np.asarray(inputs["Wk"], dtype=np.float32)
    Wv = np.asarray(inputs["Wv"], dtype=np.float32)
    m1w = np.asarray(inputs["mix1_weight"], dtype=np.float32)
    m1b = np.asarray(inputs["mix1_bias"], dtype=np.float32)
    m2w = np.asarray(inputs["mix2_weight"], dtype=np.float32)

    a1, c1 = m1w[:, 0, :], m1w[:, 1, :]
    w2 = m2w[:, :, 0]

    # layer1 stationary [17, 256]: col (half*128 + hl*16 + m)
    w1l = np.zeros((17, 256), dtype=np.float16)
    w2l = np.zeros((128, 16), dtype=np.float16)
    bcol2 = np.zeros((128, 2), dtype=np.float32)
    for h in range(H):
        half, hl = h // 8, h % 8
        for m in range(MS):
            col = half * 128 + hl * 16 + m
            w1l[h, col] = a1[h, m]
            w1l[16, col] = c1[h, m]
            w2l[hl * 16 + m, half * 8 + hl] = w2[h, m]
            bcol2[hl * 16 + m, half] = m1b[h, m]

    # padded q/k weights: head h -> 32-col slot (h%4)*32 within quad h//4
    def pad_qk(w):
        wp = np.zeros((E, 768), dtype=np.float16)
        w4 = w.reshape(E, H, D)
        for h in range(H):
            q6, sl = h // 3, h % 3
            wp[:, q6 * 128 + sl * 32:q6 * 128 + sl * 32 + 16] = w4[:, h, :]
        return wp

    def fold_eh(a):
        # [E, X] -> [128, (eh, X)]: row eh*128+p -> partition p, col-block eh
        x = a.reshape(2, 128, a.shape[1]).transpose(1, 0, 2)
        return np.ascontiguousarray(x.reshape(128, -1).astype(np.float16))

    wqk = fold_eh(np.concatenate([pad_qk(Wq), pad_qk(Wk)], axis=1))
    wv = fold_eh(Wv)

    per_core = []
    for i in range(NCORES):
        sl = slice(i * BL, (i + 1) * BL)
        xr = row_emb[sl].reshape(TOK, E).T
        xc = col_emb[sl].reshape(TOK, E).T
        per_core.append({
            "xrT": fold_eh(xr),
            "xcT": fold_eh(xc),
            "wqk": wqk,
            "wv": wv,
            "w1l": w1l,
            "w2l": w2l,
            "bcol2": bcol2,
            "cost16": cost_mat[sl].reshape(BL, PTS).astype(np.float16),
        })
    return per_core


def kernel(**inputs):
    if "nc" not in _cache:
        _cache["nc"] = build_kernel()
    nc = _cache["nc"]
    in_maps = _prep(inputs)
    res = run_bass_kernel_spmd(nc, in_maps, list(range(NCORES)))
    out = np.concatenate([res.results[i]["out"] for i in range(NCORES)],
                         axis=0)
    return out.astype(np.float32)


# revision 3
# speedup vs baseline: 1.0467x; 1.0003x over previous
"""MixedScoreMultiHeadAttention TRN2 kernel, v2.

Data-parallel over batch: 32 batches -> 8 cores x 4 batches (BL=4).

Host-side prep (free): x transposed to [E, TOK] fp16, Wq/Wk padded to
32-col head slots and packed fp16, cost flattened r-major fp16, layer1
stationary [17, 256], layer2 block-diag sign/weight matrix, bias cols.

Device pipeline per core:
  proj q/k  -> quad tiles [128, TOK] fp16 (4 head-slots x 32 rows)
  dots      -> x4all [r, (h, b, c)] fp16 (PE, per head-pair psum)
  vhat      -> [c, (h, d|1)] fp16 with ones column for softmax denom
  per b: rhs assembly (2 DMAs) -> [17, 16384] fp16 (16 dot rows + cost)
  per (b, half): 16x { 2 layer1 mm [17,128]x[17,1024] -> psum [128,1024],
                       relu+bias evac (DVE/ACT balanced) -> rr fp16,
                       8 layer2 mm rr[:,128k]x w2l -> ps2[c,(r,h)] }
                 2x exp evac [128,512] -> wsb fp16
                 16 AV mm wsb[c,r-strided] x vhat -> psa [r, (h,17)]
                 reciprocal + per-head normalize -> fout
  out DMA per b.
"""
import sys

sys.path.insert(0, "/opt/trn_rl_repo")

import numpy as np
from contextlib import ExitStack

import concourse.bass as bass
import concourse.mybir as mybir
import concourse.tile as tile
from concourse import bacc
from concourse.bass_utils import run_bass_kernel_spmd

B, R, C, E, H, D, MS = 32, 128, 128, 256, 16, 16, 16
NCORES = 8
BL = B // NCORES   # 4 batches per core
TOK = BL * R       # 512 tokens per core per side
PTS = R * C        # 16384 points per batch

FP32 = mybir.dt.float32
FP16 = mybir.dt.float16
AF = mybir.ActivationFunctionType
ALU = mybir.AluOpType


class Balancer:
    """Greedy DVE/ACT load balancing for PSUM-exit elementwise ops."""

    def __init__(self, nc):
        self.nc = nc
        self.load = {"D": 0.0, "A": 0.0}

    @staticmethod
    def _cost(eng, n, psum_src):
        if eng == "D":
            return n * 1.0417 + (125.0 if psum_src else 60.0)
        return n * 0.8333 + 185.0

    def _pick(self, n, psum_src, forbid=()):
        e = min((x for x in "DA" if x not in forbid),
                key=lambda x: self.load[x])
        self.load[e] += self._cost(e, n, psum_src)
        return e

    def charge(self, eng, n, psum_src=True):
        self.load[eng] += self._cost(eng, n, psum_src)

    def copy(self, out, in_, n):
        if self._pick(n, True) == "D":
            self.nc.vector.tensor_copy(out, in_)
        else:
            self.nc.scalar.copy(out, in_)

    def relu(self, out, in_, bias_ap, n, forbid=()):
        if self._pick(n, True, forbid) == "D":
            self.nc.vector.tensor_scalar(out, in_, bias_ap, 0.0,
                                         ALU.add, ALU.max)
        else:
            self.nc.scalar.activation(out, in_, AF.Relu, bias=bias_ap)


def build_kernel():
    nc = bacc.Bacc("TRN2", target_bir_lowering=False, debug=False,
                   num_devices=NCORES)

    xr_d = nc.dram_tensor("xrT", [128, 2 * TOK], FP16,
                          kind="ExternalInput").ap()
    xc_d = nc.dram_tensor("xcT", [128, 2 * TOK], FP16,
                          kind="ExternalInput").ap()
    wqk_d = nc.dram_tensor("wqk", [128, 2 * 2 * 768], FP16,
                           kind="ExternalInput").ap()
    wv_d = nc.dram_tensor("wv", [128, 2 * E], FP16,
                          kind="ExternalInput").ap()
    w1_d = nc.dram_tensor("w1l", [17, 256], FP16, kind="ExternalInput").ap()
    w2_d = nc.dram_tensor("w2l", [128, 16], FP16, kind="ExternalInput").ap()
    bc_d = nc.dram_tensor("bcol2", [128, 2], FP32, kind="ExternalInput").ap()
    cost_d = nc.dram_tensor("cost16", [BL, PTS], FP16,
                            kind="ExternalInput").ap()
    out_d = nc.dram_tensor("out", [BL, R, H * D], FP32,
                           kind="ExternalOutput").ap()
    # DRAM bounce for dot-score transposition, one tensor per head-quad
    # per batch-pair (b0/b1 critical-path, b2/b3 deferred)
    scr01s = [nc.dram_tensor(f"scr01_{q}", [128, 4 * 2 * C], FP16,
                             kind="Internal").ap() for q in range(4)]
    scr23s = [nc.dram_tensor(f"scr23_{q}", [128, 4 * 2 * C], FP16,
                             kind="Internal").ap() for q in range(4)]

    with tile.TileContext(nc) as tc, ExitStack() as ctx:
        const_p = ctx.enter_context(tc.tile_pool(name="const", bufs=1))
        big_p = ctx.enter_context(tc.tile_pool(name="big", bufs=1))
        rhs_p = ctx.enter_context(tc.tile_pool(name="rhs", bufs=2))
        rr_p = ctx.enter_context(tc.tile_pool(name="rr", bufs=8))
        wsb_p = ctx.enter_context(tc.tile_pool(name="wsb", bufs=4))
        fout_p = ctx.enter_context(tc.tile_pool(name="fout", bufs=1))
        small_p = ctx.enter_context(tc.tile_pool(name="small", bufs=4))
        psA = ctx.enter_context(
            tc.tile_pool(name="psA", bufs=3, space="PSUM"))   # [128,1024] x3
        ps2_p = ctx.enter_context(
            tc.tile_pool(name="ps2", bufs=2, space="PSUM"))   # [128,512] x2

        bal = Balancer(nc)

        # ---- PE warmup: ~4us of dummy matmuls on a zeroed tile so the
        # p-state ramp (2-4x slower first 3us) completes before real work
        zwarm = const_p.tile([128, 512], FP16)
        nc.gpsimd.memset(zwarm[:], 0.0)
        wps = ps2_p.tile([128, 512], FP32, tag="ps2", name="wps")
        for i in range(7):
            nc.tensor.matmul(wps[:], zwarm[:, 0:128], zwarm[:],
                             start=(i == 0), stop=(i == 6))

        # ---- const loads (single DMAs, host-prepped layouts)
        xr2 = const_p.tile([128, 2 * TOK], FP16)   # [p, (eh, tok)]
        xc2 = const_p.tile([128, 2 * TOK], FP16)
        wqk16 = const_p.tile([128, 2 * 2 * 768], FP16)  # [p, (eh, q768|k768)]
        wv16 = const_p.tile([128, 2 * E], FP16)         # [p, (eh, 256)]
        # load order: q-projection deps first so PE starts ASAP
        nc.sync.dma_start(wqk16[:, 0:1536], wqk_d[:, 0:1536])
        nc.sync.dma_start(xr2[:], xr_d[:])
        nc.sync.dma_start(wqk16[:, 1536:3072], wqk_d[:, 1536:3072])
        nc.sync.dma_start(xc2[:], xc_d[:])
        w1sb = const_p.tile([17, 256], FP16)
        nc.gpsimd.dma_start(w1sb[:], w1_d[:])
        w2sb = const_p.tile([128, 16], FP16)
        nc.gpsimd.dma_start(w2sb[:], w2_d[:])
        bc2 = const_p.tile([128, 2], FP32)
        nc.gpsimd.dma_start(bc2[:], bc_d[:])
        nc.gpsimd.dma_start(wv16[:], wv_d[:])

        # ---- prep, mh-major: proj(q,k) for mh then dots for its 4 heads,
        #      so scratch DMAs start as early as possible
        quads = {}
        x4all = big_p.tile([128, H * BL * C], FP16, name="x4all")

        def headT(proj, h):
            q6, sl = h // 3, h % 3
            t = quads[(proj, q6 // 2)]
            co = (q6 % 2) * TOK
            return t[sl * 32:sl * 32 + 16, co:co + TOK]

        # rhs tiles for b0/b1 up front; cost rows load immediately, dot
        # rows arrive as quad-sized stage2 DMAs pipelined into the mh loop
        # views [r, h4, b2, c] for the stage2 per-batch slices
        s01v = [sd[:].rearrange("r (h b c) -> r h b c", h=4, b=2)
                for sd in scr01s]
        s23v = [sd[:].rearrange("r (h b c) -> r h b c", h=4, b=2)
                for sd in scr23s]
        rhs0 = rhs_p.tile([17, PTS], FP16, tag="rhs", name="rhs0")
        rhs1 = rhs_p.tile([17, PTS], FP16, tag="rhs", name="rhs1")
        nc.scalar.dma_start(rhs0[16:17, :], cost_d[0:1, :])
        nc.scalar.dma_start(rhs1[16:17, :], cost_d[1:2, :])

        def emit_proj(pt):
            # pairtile pt covers 6-quads 2pt, 2pt+1 (3 heads each)
            for proj, qofs in (("q", 0), ("k", 768)):
                ps = psA.tile([128, 1024], FP32, tag="psA", name="ps")
                for qp in range(2):
                    q6 = pt * 2 + qp
                    for eh in range(2):
                        nc.tensor.matmul(
                            ps[0:96, qp * TOK:(qp + 1) * TOK],
                            wqk16[:, eh * 1536 + qofs + q6 * 128:
                                  eh * 1536 + qofs + q6 * 128 + 96],
                            xr2[:, eh * TOK:(eh + 1) * TOK] if proj == "q"
                            else xc2[:, eh * TOK:(eh + 1) * TOK],
                            start=(eh == 0), stop=(eh == 1))
                qt = big_p.tile([96, 2 * TOK], FP16, tag=f"{proj}T{pt}",
                                name=f"{proj}T{pt}")
                bal.copy(qt[:], ps[0:96, :], 1024)
                quads[(proj, pt)] = qt

        vhats = {}
        vh_tiles = {}
        for b in range(BL):
            vh = big_p.tile([128, 17 * H], FP16, tag=f"vhat{b}",
                            name=f"vhat{b}")
            vh_tiles[b] = vh
            nc.gpsimd.memset(
                vh[:].rearrange("p (h x) -> p h x", h=H)[:, :, 16:17], 1.0)

        def emit_vhat(b):
            vh = vh_tiles[b]
            vh3 = vh[:].rearrange("p (h x) -> p h x", h=H)
            ps = psA.tile([128, 1024], FP32, tag="psA", name="ps")
            for eh in range(2):
                nc.tensor.matmul(
                    ps[:, 0:E],
                    xc2[:, eh * TOK + b * 128:eh * TOK + (b + 1) * 128],
                    wv16[:, eh * E:(eh + 1) * E],
                    start=(eh == 0), stop=(eh == 1))
            bal.copy(vh3[:, :, 0:16],
                     ps[:, 0:E].rearrange("p (h x) -> p h x", h=H), E)
            vhats[b] = vh

        # proj runs 2 mh ahead of dots; vhat interleaved to fill gaps
        emit_proj(0)
        emit_proj(1)
        for mh in range(4):
            for hp2 in range(2):   # dots for heads 4mh .. 4mh+3
                ps = psA.tile([128, 1024], FP32, tag="psA", name="ps")
                for hh in range(2):
                    h = mh * 4 + hp2 * 2 + hh
                    for b in range(BL):
                        nc.tensor.matmul(
                            ps[:, hh * 512 + b * 128:hh * 512 + (b + 1) * 128],
                            headT("q", h)[:, b * 128:(b + 1) * 128],
                            headT("k", h)[:, b * 128:(b + 1) * 128])
                hbase = mh * 4 + hp2 * 2
                bal.copy(x4all[:, hbase * 512:(hbase + 2) * 512], ps[:], 1024)
            # stage1 for b0/b1 only (first 256 cols of each h-block)
            nc.sync.dma_start(
                scr01s[mh][:],
                x4all[:].rearrange("r (h x) -> r h x", h=H)
                [:, mh * 4:(mh + 1) * 4, 0:256])
            # stage2 quad DMAs: b0 on SP behind its stage1, b1 on Pool
            for eng, bb, rhsq in ((nc.sync, 0, rhs0), (nc.gpsimd, 1, rhs1)):
                eng.dma_start(
                    rhsq[mh * 4:(mh + 1) * 4, :].rearrange(
                        "h (r c) -> h r c", c=C),
                    s01v[mh][:, :, bb, :].rearrange("r h c -> h r c"))
            if mh == 1:
                emit_proj(2)

        for b in range(BL):
            emit_vhat(b)

        # ---- per-batch MLP + softmax + AV
        fouts = [fout_p.tile([128, H * D], FP32, tag=f"fo{b}", name=f"fo{b}")
                 for b in range(BL)]

        # deferred stage1 for b2/b3 (off the critical path)
        for q in range(4):
            nc.sync.dma_start(
                scr23s[q][:],
                x4all[:].rearrange("r (h x) -> r h x", h=H)
                [:, q * 4:(q + 1) * 4, 256:512])

        def assemble(b):
            rhs = rhs_p.tile([17, PTS], FP16, tag="rhs", name="rhs")
            # dots rows: dst (h; r, c) <- scratch [h, r, b, c] slice
            for q in range(4):
                nc.sync.dma_start(
                    rhs[q * 4:(q + 1) * 4, :].rearrange(
                        "h (r c) -> h r c", c=C),
                    s23v[q][:, :, b - 2, :].rearrange("r h c -> h r c"))
            nc.sync.dma_start(rhs[16:17, :], cost_d[b:b + 1, :])
            return rhs

        def make_tail(b, half, wsbs):
            # AV + normalize for (b, half), deferred into the next half's
            # pair loop so the PE never stalls on the ACT-queued exp
            def tail():
                psa = ps2_p.tile([128, 17 * 8], FP32, tag="ps2", name="psa")
                for grp in range(2):
                    w4 = wsbs[grp][:].rearrange("p (r h) -> p r h", h=8)
                    for hl in range(8):
                        h = half * 8 + hl
                        nc.tensor.matmul(
                            psa[grp * 64:(grp + 1) * 64,
                                hl * 17:hl * 17 + 17],
                            w4[:, :, hl],
                            vhats[b][:, h * 17:(h + 1) * 17])
                psa3 = psa[:].rearrange("p (x y) -> p x y", x=8)
                rec = small_p.tile([128, 8], FP32, tag="rec")
                nc.vector.reciprocal(rec[:], psa3[:, :, 16])
                bal.charge("D", 8)
                recb = rec[:].rearrange(
                    "p (h o) -> p h o", o=1).broadcast_to([128, 8, 16])
                nc.vector.tensor_tensor(
                    fouts[b][:, half * 128:(half + 1) * 128].rearrange(
                        "p (h x) -> p h x", h=8),
                    psa3[:, :, 0:16], recb, ALU.mult)
                bal.charge("D", 128)
                if half == 1:
                    nc.sync.dma_start(out_d[b], fouts[b][:])
            return tail

        rhss = {0: rhs0, 1: rhs1}
        prev_tail = None
        for b in range(BL):
            if b + 2 < BL:
                rhss[b + 2] = assemble(b + 2)
            rhs = rhss.pop(b)
            for half in range(2):
                w1h = w1sb[:, half * 128:(half + 1) * 128]
                w2h = w2sb[:, half * 8:(half + 1) * 8]
                bch = bc2[:, half:half + 1]
                ps2s = []
                pend = []   # (rr, pair) with layer2 not yet emitted

                def emit_l2(rr, pair, ps2s=ps2s, w2h=w2h):
                    if pair % 8 == 0:
                        ps2t = ps2_p.tile([128, 512], FP32, tag="ps2",
                                          name="ps2t")
                        ps2s.append(ps2t)
                    for s in range(8):
                        rloc = (pair % 8) * 8 + s
                        nc.tensor.matmul(
                            ps2s[-1][:, rloc * 8:rloc * 8 + 8],
                            rr[:, s * 128:(s + 1) * 128], w2h)

                wsbs = []
                for pair in range(16):
                    ps1 = psA.tile([128, 1024], FP32, tag="psA")
                    for k in range(2):
                        ck = pair * 2 + k
                        nc.tensor.matmul(
                            ps1[:, k * 512:(k + 1) * 512], w1h,
                            rhs[:, ck * 512:(ck + 1) * 512])
                    rr = rr_p.tile([128, 1024], FP16, tag="rr")
                    bal.relu(rr[:], ps1[:], bch, 1024)
                    pend.append((rr, pair))
                    if len(pend) > 4:
                        emit_l2(*pend.pop(0))
                    if pair == 4 and prev_tail is not None:
                        prev_tail()
                        prev_tail = None
                    # grp0 fully written once l2(p7) emitted -> exp it early
                    if pend and pend[0][1] == 8 and len(ps2s) == 1:
                        wsb = wsb_p.tile([128, 512], FP16, tag="wsb")
                        nc.scalar.activation(wsb[:], ps2s[0][:], AF.Exp)
                        bal.charge("A", 512)
                        wsbs.append(wsb)
                while pend:
                    emit_l2(*pend.pop(0))
                wsb = wsb_p.tile([128, 512], FP16, tag="wsb")
                nc.scalar.activation(wsb[:], ps2s[1][:], AF.Exp)
                bal.charge("A", 512)
                wsbs.append(wsb)
                prev_tail = make_tail(b, half, wsbs)

        prev_tail()

    nc.compile()
    return nc


_cache = {}


def _prep(inputs):
    row_emb = np.asarray(inputs["row_emb"], dtype=np.float32)
    col_emb = np.asarray(inputs["col_emb"], dtype=np.float32)
    cost_mat = np.asarray(inputs["cost_mat"], dtype=np.float32)
    Wq = np.asarray(inputs["Wq"], dtype=np.float32) / np.sqrt(D)
    Wk = np.asarray(inputs["Wk"], dtype=np.float32)
    Wv = np.asarray(inputs["Wv"], dtype=np.float32)
    m1w = np.asarray(inputs["mix1_weight"], dtype=np.float32)
    m1b = np.asarray(inputs["mix1_bias"], dtype=np.float32)
    m2w = np.asarray(inputs["mix2_weight"], dtype=np.float32)

    a1, c1 = m1w[:, 0, :], m1w[:, 1, :]
    w2 = m2w[:, :, 0]

    # layer1 stationary [17, 256]: col (half*128 + hl*16 + m)
    w1l = np.zeros((17, 256), dtype=np.float16)
    w2l = np.zeros((128, 16), dtype=np.float16)
    bcol2 = np.zeros((128, 2), dtype=np.float32)
    for h in range(H):
        half, hl = h // 8, h % 8
        for m in range(MS):
            col = half * 128 + hl * 16 + m
            w1l[h, col] = a1[h, m]
            w1l[16, col] = c1[h, m]
            w2l[hl * 16 + m, half * 8 + hl] = w2[h, m]
            bcol2[hl * 16 + m, half] = m1b[h, m]

    # padded q/k weights: head h -> 32-col slot (h%4)*32 within quad h//4
    def pad_qk(w):
        wp = np.zeros((E, 768), dtype=np.float16)
        w4 = w.reshape(E, H, D)
        for h in range(H):
            q6, sl = h // 3, h % 3
            wp[:, q6 * 128 + sl * 32:q6 * 128 + sl * 32 + 16] = w4[:, h, :]
        return wp

    def fold_eh(a):
        # [E, X] -> [128, (eh, X)]: row eh*128+p -> partition p, col-block eh
        x = a.reshape(2, 128, a.shape[1]).transpose(1, 0, 2)
        return np.ascontiguousarray(x.reshape(128, -1).astype(np.float16))

    wqk = fold_eh(np.concatenate([pad_qk(Wq), pad_qk(Wk)], axis=1))
    wv = fold_eh(Wv)

    per_core = []
    for i in range(NCORES):
        sl = slice(i * BL, (i + 1) * BL)
        xr = row_emb[sl].reshape(TOK, E).T
        xc = col_emb[sl].reshape(TOK, E).T
        per_core.append({
            "xrT": fold_eh(xr),
            "xcT": fold_eh(xc),
            "wqk": wqk,
            "wv": wv,
            "w1l": w1l,
            "w2l": w2l,
            "bcol2": bcol2,
            "cost16": cost_mat[sl].reshape(BL, PTS).astype(np.float16),
        })
    return per_core


def kernel(**inputs):
    if "nc" not in _cache:
        _cache["nc"] = build_kernel()
    nc = _cache["nc"]
    in_maps = _prep(inputs)
    res = run_bass_kernel_spmd(nc, in_maps, list(range(NCORES)))
    out = np.concatenate([res.results[i]["out"] for i in range(NCORES)],
                         axis=0)
    return out.astype(np.float32)


# revision 4
# speedup vs baseline: 1.0727x; 1.0248x over previous
"""MixedScoreMultiHeadAttention TRN2 kernel, v2.

Data-parallel over batch: 32 batches -> 8 cores x 4 batches (BL=4).

Host-side prep (free): x transposed to [E, TOK] fp16, Wq/Wk padded to
32-col head slots and packed fp16, cost flattened r-major fp16, layer1
stationary [17, 256], layer2 block-diag sign/weight matrix, bias cols.

Device pipeline per core:
  proj q/k  -> quad tiles [128, TOK] fp16 (4 head-slots x 32 rows)
  dots      -> x4all [r, (h, b, c)] fp16 (PE, per head-pair psum)
  vhat      -> [c, (h, d|1)] fp16 with ones column for softmax denom
  per b: rhs assembly (2 DMAs) -> [17, 16384] fp16 (16 dot rows + cost)
  per (b, half): 16x { 2 layer1 mm [17,128]x[17,1024] -> psum [128,1024],
                       relu+bias evac (DVE/ACT balanced) -> rr fp16,
                       8 layer2 mm rr[:,128k]x w2l -> ps2[c,(r,h)] }
                 2x exp evac [128,512] -> wsb fp16
                 16 AV mm wsb[c,r-strided] x vhat -> psa [r, (h,17)]
                 reciprocal + per-head normalize -> fout
  out DMA per b.
"""
import sys

sys.path.insert(0, "/opt/trn_rl_repo")

import numpy as np
from contextlib import ExitStack

import concourse.bass as bass
import concourse.mybir as mybir
import concourse.tile as tile
from concourse import bacc
from concourse.bass_utils import run_bass_kernel_spmd

B, R, C, E, H, D, MS = 32, 128, 128, 256, 16, 16, 16
NCORES = 8
BL = B // NCORES   # 4 batches per core
TOK = BL * R       # 512 tokens per core per side
PTS = R * C        # 16384 points per batch

FP32 = mybir.dt.float32
FP16 = mybir.dt.float16
AF = mybir.ActivationFunctionType
ALU = mybir.AluOpType


class Balancer:
    """Greedy DVE/ACT load balancing for PSUM-exit elementwise ops."""

    def __init__(self, nc):
        self.nc = nc
        self.load = {"D": 0.0, "A": 0.0}

    @staticmethod
    def _cost(eng, n, psum_src):
        if eng == "D":
            return n * 1.0417 + (125.0 if psum_src else 60.0)
        return n * 0.8333 + 185.0

    def _pick(self, n, psum_src, forbid=()):
        e = min((x for x in "DA" if x not in forbid),
                key=lambda x: self.load[x])
        self.load[e] += self._cost(e, n, psum_src)
        return e

    def charge(self, eng, n, psum_src=True):
        self.load[eng] += self._cost(eng, n, psum_src)

    def copy(self, out, in_, n):
        self._ci = getattr(self, "_ci", 0) + 1
        if self._ci % 2 == 1:
            self.charge("D", n)
            self.nc.vector.tensor_copy(out, in_)
        else:
            self.charge("A", n)
            self.nc.scalar.copy(out, in_)

    def relu(self, out, in_, bias_ap, n, forbid=()):
        if self._pick(n, True, forbid) == "D":
            self.nc.vector.tensor_scalar(out, in_, bias_ap, 0.0,
                                         ALU.add, ALU.max)
        else:
            self.nc.scalar.activation(out, in_, AF.Relu, bias=bias_ap)


def build_kernel():
    nc = bacc.Bacc("TRN2", target_bir_lowering=False, debug=False,
                   num_devices=NCORES)

    xr_d = nc.dram_tensor("xrT", [128, 2 * TOK], FP16,
                          kind="ExternalInput").ap()
    xc_d = nc.dram_tensor("xcT", [128, 2 * TOK], FP16,
                          kind="ExternalInput").ap()
    wqk_d = nc.dram_tensor("wqk", [128, 2 * 2 * 768], FP16,
                           kind="ExternalInput").ap()
    wv_d = nc.dram_tensor("wv", [128, 2 * E], FP16,
                          kind="ExternalInput").ap()
    w1_d = nc.dram_tensor("w1l", [17, 256], FP16, kind="ExternalInput").ap()
    w2_d = nc.dram_tensor("w2l", [128, 16], FP16, kind="ExternalInput").ap()
    bc_d = nc.dram_tensor("bcol2", [128, 2], FP32, kind="ExternalInput").ap()
    cost_d = nc.dram_tensor("cost16", [BL, PTS], FP16,
                            kind="ExternalInput").ap()
    out_d = nc.dram_tensor("out", [BL, R, H * D], FP32,
                           kind="ExternalOutput").ap()
    # DRAM bounce for dot-score transposition, one tensor per head-quad
    # per batch-pair (b0/b1 critical-path, b2/b3 deferred)
    scr01s = [nc.dram_tensor(f"scr01_{q}", [128, 4 * 2 * C], FP16,
                             kind="Internal").ap() for q in range(4)]
    scr23s = [nc.dram_tensor(f"scr23_{q}", [128, 4 * 2 * C], FP16,
                             kind="Internal").ap() for q in range(4)]

    with tile.TileContext(nc) as tc, ExitStack() as ctx:
        const_p = ctx.enter_context(tc.tile_pool(name="const", bufs=1))
        big_p = ctx.enter_context(tc.tile_pool(name="big", bufs=1))
        rhs_p = ctx.enter_context(tc.tile_pool(name="rhs", bufs=2))
        rr_p = ctx.enter_context(tc.tile_pool(name="rr", bufs=8))
        wsb_p = ctx.enter_context(tc.tile_pool(name="wsb", bufs=4))
        fout_p = ctx.enter_context(tc.tile_pool(name="fout", bufs=1))
        small_p = ctx.enter_context(tc.tile_pool(name="small", bufs=4))
        psA = ctx.enter_context(
            tc.tile_pool(name="psA", bufs=3, space="PSUM"))   # [128,1024] x3
        ps2_p = ctx.enter_context(
            tc.tile_pool(name="ps2", bufs=2, space="PSUM"))   # [128,512] x2

        bal = Balancer(nc)

        # ---- PE warmup: ~4us of dummy matmuls on a zeroed tile so the
        # p-state ramp (2-4x slower first 3us) completes before real work
        zwarm = const_p.tile([128, 512], FP16)
        nc.gpsimd.memset(zwarm[:], 0.0)
        wps = ps2_p.tile([128, 512], FP32, tag="ps2", name="wps")
        for i in range(7):
            nc.tensor.matmul(wps[:], zwarm[:, 0:128], zwarm[:],
                             start=(i == 0), stop=(i == 6))

        # ---- const loads (single DMAs, host-prepped layouts)
        xr2 = const_p.tile([128, 2 * TOK], FP16)   # [p, (eh, tok)]
        xc2 = const_p.tile([128, 2 * TOK], FP16)
        wqk16 = const_p.tile([128, 2 * 2 * 768], FP16)  # [p, (eh, q768|k768)]
        wv16 = const_p.tile([128, 2 * E], FP16)         # [p, (eh, 256)]
        # load order: q-projection deps first so PE starts ASAP
        nc.sync.dma_start(wqk16[:, 0:1536], wqk_d[:, 0:1536])
        nc.sync.dma_start(xr2[:], xr_d[:])
        nc.sync.dma_start(wqk16[:, 1536:3072], wqk_d[:, 1536:3072])
        nc.sync.dma_start(xc2[:], xc_d[:])
        w1sb = const_p.tile([17, 256], FP16)
        nc.gpsimd.dma_start(w1sb[:], w1_d[:])
        w2sb = const_p.tile([128, 16], FP16)
        nc.gpsimd.dma_start(w2sb[:], w2_d[:])
        bc2 = const_p.tile([128, 2], FP32)
        nc.gpsimd.dma_start(bc2[:], bc_d[:])
        nc.gpsimd.dma_start(wv16[:], wv_d[:])

        # ---- prep, mh-major: proj(q,k) for mh then dots for its 4 heads,
        #      so scratch DMAs start as early as possible
        quads = {}
        x4all = big_p.tile([128, H * BL * C], FP16, name="x4all")

        def headT(proj, h):
            q6, sl = h // 3, h % 3
            t = quads[(proj, q6 // 2)]
            co = (q6 % 2) * TOK
            return t[sl * 32:sl * 32 + 16, co:co + TOK]

        # rhs tiles for b0/b1 up front; cost rows load immediately, dot
        # rows arrive as quad-sized stage2 DMAs pipelined into the mh loop
        # views [r, h4, b2, c] for the stage2 per-batch slices
        s01v = [sd[:].rearrange("r (h b c) -> r h b c", h=4, b=2)
                for sd in scr01s]
        s23v = [sd[:].rearrange("r (h b c) -> r h b c", h=4, b=2)
                for sd in scr23s]
        rhs0 = rhs_p.tile([17, PTS], FP16, tag="rhs", name="rhs0")
        rhs1 = rhs_p.tile([17, PTS], FP16, tag="rhs", name="rhs1")
        nc.scalar.dma_start(rhs0[16:17, :], cost_d[0:1, :])
        nc.scalar.dma_start(rhs1[16:17, :], cost_d[1:2, :])

        def emit_proj(pt):
            # pairtile pt covers 6-quads 2pt, 2pt+1 (3 heads each)
            for proj, qofs in (("q", 0), ("k", 768)):
                ps = psA.tile([128, 1024], FP32, tag="psA", name="ps")
                for qp in range(2):
                    q6 = pt * 2 + qp
                    for eh in range(2):
                        nc.tensor.matmul(
                            ps[0:96, qp * TOK:(qp + 1) * TOK],
                            wqk16[:, eh * 1536 + qofs + q6 * 128:
                                  eh * 1536 + qofs + q6 * 128 + 96],
                            xr2[:, eh * TOK:(eh + 1) * TOK] if proj == "q"
                            else xc2[:, eh * TOK:(eh + 1) * TOK],
                            start=(eh == 0), stop=(eh == 1))
                qt = big_p.tile([96, 2 * TOK], FP16, tag=f"{proj}T{pt}",
                                name=f"{proj}T{pt}")
                bal.copy(qt[:], ps[0:96, :], 1024)
                quads[(proj, pt)] = qt

        vhats = {}
        vh_tiles = {}
        for b in range(BL):
            vh = big_p.tile([128, 17 * H], FP16, tag=f"vhat{b}",
                            name=f"vhat{b}")
            vh_tiles[b] = vh
            nc.gpsimd.memset(
                vh[:].rearrange("p (h x) -> p h x", h=H)[:, :, 16:17], 1.0)

        def emit_vhat(b):
            vh = vh_tiles[b]
            vh3 = vh[:].rearrange("p (h x) -> p h x", h=H)
            ps = psA.tile([128, 1024], FP32, tag="psA", name="ps")
            for eh in range(2):
                nc.tensor.matmul(
                    ps[:, 0:E],
                    xc2[:, eh * TOK + b * 128:eh * TOK + (b + 1) * 128],
                    wv16[:, eh * E:(eh + 1) * E],
                    start=(eh == 0), stop=(eh == 1))
            bal.copy(vh3[:, :, 0:16],
                     ps[:, 0:E].rearrange("p (h x) -> p h x", h=H), E)
            vhats[b] = vh

        # proj runs 2 mh ahead of dots; vhat interleaved to fill gaps
        emit_proj(0)
        emit_proj(1)
        for mh in range(4):
            for hp2 in range(2):   # dots for heads 4mh .. 4mh+3
                ps = psA.tile([128, 1024], FP32, tag="psA", name="ps")
                for hh in range(2):
                    h = mh * 4 + hp2 * 2 + hh
                    for b in range(BL):
                        nc.tensor.matmul(
                            ps[:, hh * 512 + b * 128:hh * 512 + (b + 1) * 128],
                            headT("q", h)[:, b * 128:(b + 1) * 128],
                            headT("k", h)[:, b * 128:(b + 1) * 128])
                hbase = mh * 4 + hp2 * 2
                bal.copy(x4all[:, hbase * 512:(hbase + 2) * 512], ps[:], 1024)
            # stage1 for b0/b1 only (first 256 cols of each h-block)
            nc.sync.dma_start(
                scr01s[mh][:],
                x4all[:].rearrange("r (h x) -> r h x", h=H)
                [:, mh * 4:(mh + 1) * 4, 0:256])
            # stage2 quad DMAs: b0 on SP behind its stage1, b1 on Pool
            for eng, bb, rhsq in ((nc.sync, 0, rhs0), (nc.gpsimd, 1, rhs1)):
                eng.dma_start(
                    rhsq[mh * 4:(mh + 1) * 4, :].rearrange(
                        "h (r c) -> h r c", c=C),
                    s01v[mh][:, :, bb, :].rearrange("r h c -> h r c"))
            if mh == 1:
                emit_proj(2)

        for b in range(BL):
            emit_vhat(b)

        # ---- per-batch MLP + softmax + AV
        fouts = [fout_p.tile([128, H * D], FP32, tag=f"fo{b}", name=f"fo{b}")
                 for b in range(BL)]

        # deferred stage1 for b2/b3 (off the critical path)
        for q in range(4):
            nc.sync.dma_start(
                scr23s[q][:],
                x4all[:].rearrange("r (h x) -> r h x", h=H)
                [:, q * 4:(q + 1) * 4, 256:512])

        def assemble(b):
            rhs = rhs_p.tile([17, PTS], FP16, tag="rhs", name="rhs")
            # dots rows: dst (h; r, c) <- scratch [h, r, b, c] slice
            for q in range(4):
                nc.sync.dma_start(
                    rhs[q * 4:(q + 1) * 4, :].rearrange(
                        "h (r c) -> h r c", c=C),
                    s23v[q][:, :, b - 2, :].rearrange("r h c -> h r c"))
            nc.sync.dma_start(rhs[16:17, :], cost_d[b:b + 1, :])
            return rhs

        def make_tail(b, half, wsbs):
            # AV + normalize for (b, half), deferred into the next half's
            # pair loop so the PE never stalls on the ACT-queued exp
            def tail():
                psa = ps2_p.tile([128, 17 * 8], FP32, tag="ps2", name="psa")
                for grp in range(2):
                    w4 = wsbs[grp][:].rearrange("p (r h) -> p r h", h=8)
                    for hl in range(8):
                        h = half * 8 + hl
                        nc.tensor.matmul(
                            psa[grp * 64:(grp + 1) * 64,
                                hl * 17:hl * 17 + 17],
                            w4[:, :, hl],
                            vhats[b][:, h * 17:(h + 1) * 17])
                psa3 = psa[:].rearrange("p (x y) -> p x y", x=8)
                rec = small_p.tile([128, 8], FP32, tag="rec")
                nc.vector.reciprocal(rec[:], psa3[:, :, 16])
                bal.charge("D", 8)
                recb = rec[:].rearrange(
                    "p (h o) -> p h o", o=1).broadcast_to([128, 8, 16])
                nc.vector.tensor_tensor(
                    fouts[b][:, half * 128:(half + 1) * 128].rearrange(
                        "p (h x) -> p h x", h=8),
                    psa3[:, :, 0:16], recb, ALU.mult)
                bal.charge("D", 128)
                if half == 1:
                    nc.sync.dma_start(out_d[b], fouts[b][:])
            return tail

        rhss = {0: rhs0, 1: rhs1}
        prev_tail = None
        for b in range(BL):
            if b + 2 < BL:
                rhss[b + 2] = assemble(b + 2)
            rhs = rhss.pop(b)
            for half in range(2):
                w1h = w1sb[:, half * 128:(half + 1) * 128]
                w2h = w2sb[:, half * 8:(half + 1) * 8]
                bch = bc2[:, half:half + 1]
                ps2s = []
                pend = []   # (rr, pair) with layer2 not yet emitted

                def emit_l2(rr, pair, ps2s=ps2s, w2h=w2h):
                    if pair % 8 == 0:
                        ps2t = ps2_p.tile([128, 512], FP32, tag="ps2",
                                          name="ps2t")
                        ps2s.append(ps2t)
                    for s in range(8):
                        rloc = (pair % 8) * 8 + s
                        nc.tensor.matmul(
                            ps2s[-1][:, rloc * 8:rloc * 8 + 8],
                            rr[:, s * 128:(s + 1) * 128], w2h)

                wsbs = []
                for pair in range(16):
                    ps1 = psA.tile([128, 1024], FP32, tag="psA")
                    for k in range(2):
                        ck = pair * 2 + k
                        nc.tensor.matmul(
                            ps1[:, k * 512:(k + 1) * 512], w1h,
                            rhs[:, ck * 512:(ck + 1) * 512])
                    rr = rr_p.tile([128, 1024], FP16, tag="rr")
                    if pair % 2 == 0:
                        nc.vector.tensor_scalar(rr[:], ps1[:], bch, 0.0,
                                                ALU.add, ALU.max)
                        bal.charge("D", 1024)
                    else:
                        nc.scalar.activation(rr[:], ps1[:], AF.Relu, bias=bch)
                        bal.charge("A", 1024)
                    pend.append((rr, pair))
                    if len(pend) > 4:
                        emit_l2(*pend.pop(0))
                    if pair == 4 and prev_tail is not None:
                        prev_tail()
                        prev_tail = None
                    # grp0 fully written once l2(p7) emitted -> exp it early
                    if pend and pend[0][1] == 8 and len(ps2s) == 1:
                        wsb = wsb_p.tile([128, 512], FP16, tag="wsb")
                        nc.scalar.activation(wsb[:], ps2s[0][:], AF.Exp)
                        bal.charge("A", 512)
                        wsbs.append(wsb)
                while pend:
                    emit_l2(*pend.pop(0))
                wsb = wsb_p.tile([128, 512], FP16, tag="wsb")
                nc.scalar.activation(wsb[:], ps2s[1][:], AF.Exp)
                bal.charge("A", 512)
                wsbs.append(wsb)
                prev_tail = make_tail(b, half, wsbs)

        prev_tail()

    nc.compile()
    return nc


_cache = {}


def _prep(inputs):
    row_emb = np.asarray(inputs["row_emb"], dtype=np.float32)
    col_emb = np.asarray(inputs["col_emb"], dtype=np.float32)
    cost_mat = np.asarray(inputs["cost_mat"], dtype=np.float32)
    Wq = np.asarray(inputs["Wq"], dtype=np.float32) / np.sqrt(D)
    Wk = np.asarray(inputs["Wk"], dtype=np.float32)
    Wv = np.asarray(inputs["Wv"], dtype=np.float32)
    m1w = np.asarray(inputs["mix1_weight"], dtype=np.float32)
    m1b = np.asarray(inputs["mix1_bias"], dtype=np.float32)
    m2w = np.asarray(inputs["mix2_weight"], dtype=np.float32)

    a1, c1 = m1w[:, 0, :], m1w[:, 1, :]
    w2 = m2w[:, :, 0]

    # layer1 stationary [17, 256]: col (half*128 + hl*16 + m)
    w1l = np.zeros((17, 256), dtype=np.float16)
    w2l = np.zeros((128, 16), dtype=np.float16)
    bcol2 = np.zeros((128, 2), dtype=np.float32)
    for h in range(H):
        half, hl = h // 8, h % 8
        for m in range(MS):
            col = half * 128 + hl * 16 + m
            w1l[h, col] = a1[h, m]
            w1l[16, col] = c1[h, m]
            w2l[hl * 16 + m, half * 8 + hl] = w2[h, m]
            bcol2[hl * 16 + m, half] = m1b[h, m]

    # padded q/k weights: head h -> 32-col slot (h%4)*32 within quad h//4
    def pad_qk(w):
        wp = np.zeros((E, 768), dtype=np.float16)
        w4 = w.reshape(E, H, D)
        for h in range(H):
            q6, sl = h // 3, h % 3
            wp[:, q6 * 128 + sl * 32:q6 * 128 + sl * 32 + 16] = w4[:, h, :]
        return wp

    def fold_eh(a):
        # [E, X] -> [128, (eh, X)]: row eh*128+p -> partition p, col-block eh
        x = a.reshape(2, 128, a.shape[1]).transpose(1, 0, 2)
        return np.ascontiguousarray(x.reshape(128, -1).astype(np.float16))

    wqk = fold_eh(np.concatenate([pad_qk(Wq), pad_qk(Wk)], axis=1))
    wv = fold_eh(Wv)

    per_core = []
    for i in range(NCORES):
        sl = slice(i * BL, (i + 1) * BL)
        xr = row_emb[sl].reshape(TOK, E).T
        xc = col_emb[sl].reshape(TOK, E).T
        per_core.append({
            "xrT": fold_eh(xr),
            "xcT": fold_eh(xc),
            "wqk": wqk,
            "wv": wv,
            "w1l": w1l,
            "w2l": w2l,
            "bcol2": bcol2,
            "cost16": cost_mat[sl].reshape(BL, PTS).astype(np.float16),
        })
    return per_core


def kernel(**inputs):
    if "nc" not in _cache:
        _cache["nc"] = build_kernel()
    nc = _cache["nc"]
    in_maps = _prep(inputs)
    res = run_bass_kernel_spmd(nc, in_maps, list(range(NCORES)))
    out = np.concatenate([res.results[i]["out"] for i in range(NCORES)],
                         axis=0)
    return out.astype(np.float32)


# revision 5
# speedup vs baseline: 1.0753x; 1.0025x over previous
"""MixedScoreMultiHeadAttention TRN2 kernel, v2.

Data-parallel over batch: 32 batches -> 8 cores x 4 batches (BL=4).

Host-side prep (free): x transposed to [E, TOK] fp16, Wq/Wk padded to
32-col head slots and packed fp16, cost flattened r-major fp16, layer1
stationary [17, 256], layer2 block-diag sign/weight matrix, bias cols.

Device pipeline per core:
  proj q/k  -> quad tiles [128, TOK] fp16 (4 head-slots x 32 rows)
  dots      -> x4all [r, (h, b, c)] fp16 (PE, per head-pair psum)
  vhat      -> [c, (h, d|1)] fp16 with ones column for softmax denom
  per b: rhs assembly (2 DMAs) -> [17, 16384] fp16 (16 dot rows + cost)
  per (b, half): 16x { 2 layer1 mm [17,128]x[17,1024] -> psum [128,1024],
                       relu+bias evac (DVE/ACT balanced) -> rr fp16,
                       8 layer2 mm rr[:,128k]x w2l -> ps2[c,(r,h)] }
                 2x exp evac [128,512] -> wsb fp16
                 16 AV mm wsb[c,r-strided] x vhat -> psa [r, (h,17)]
                 reciprocal + per-head normalize -> fout
  out DMA per b.
"""
import sys

sys.path.insert(0, "/opt/trn_rl_repo")

import numpy as np
from contextlib import ExitStack

import concourse.bass as bass
import concourse.mybir as mybir
import concourse.tile as tile
from concourse import bacc
from concourse.bass_utils import run_bass_kernel_spmd

B, R, C, E, H, D, MS = 32, 128, 128, 256, 16, 16, 16
NCORES = 8
BL = B // NCORES   # 4 batches per core
TOK = BL * R       # 512 tokens per core per side
PTS = R * C        # 16384 points per batch

FP32 = mybir.dt.float32
FP16 = mybir.dt.float16
AF = mybir.ActivationFunctionType
ALU = mybir.AluOpType


class Balancer:
    """Greedy DVE/ACT load balancing for PSUM-exit elementwise ops."""

    def __init__(self, nc):
        self.nc = nc
        self.load = {"D": 0.0, "A": 0.0}

    @staticmethod
    def _cost(eng, n, psum_src):
        if eng == "D":
            return n * 1.0417 + (125.0 if psum_src else 60.0)
        return n * 0.8333 + 185.0

    def _pick(self, n, psum_src, forbid=()):
        e = min((x for x in "DA" if x not in forbid),
                key=lambda x: self.load[x])
        self.load[e] += self._cost(e, n, psum_src)
        return e

    def charge(self, eng, n, psum_src=True):
        self.load[eng] += self._cost(eng, n, psum_src)

    def copy(self, out, in_, n):
        self._ci = getattr(self, "_ci", 0) + 1
        if self._ci % 2 == 1:
            self.charge("D", n)
            self.nc.vector.tensor_copy(out, in_)
        else:
            self.charge("A", n)
            self.nc.scalar.copy(out, in_)

    def relu(self, out, in_, bias_ap, n, forbid=()):
        if self._pick(n, True, forbid) == "D":
            self.nc.vector.tensor_scalar(out, in_, bias_ap, 0.0,
                                         ALU.add, ALU.max)
        else:
            self.nc.scalar.activation(out, in_, AF.Relu, bias=bias_ap)


def build_kernel():
    nc = bacc.Bacc("TRN2", target_bir_lowering=False, debug=False,
                   num_devices=NCORES)

    xr_d = nc.dram_tensor("xrT", [128, 2 * TOK], FP16,
                          kind="ExternalInput").ap()
    xc_d = nc.dram_tensor("xcT", [128, 2 * TOK], FP16,
                          kind="ExternalInput").ap()
    wqk_d = nc.dram_tensor("wqk", [128, 2 * 2 * 768], FP16,
                           kind="ExternalInput").ap()
    wv_d = nc.dram_tensor("wv", [128, 2 * E], FP16,
                          kind="ExternalInput").ap()
    w1_d = nc.dram_tensor("w1l", [17, 256], FP16, kind="ExternalInput").ap()
    w2_d = nc.dram_tensor("w2l", [128, 16], FP16, kind="ExternalInput").ap()
    bc_d = nc.dram_tensor("bcol2", [128, 2], FP32, kind="ExternalInput").ap()
    cost_d = nc.dram_tensor("cost16", [BL, PTS], FP16,
                            kind="ExternalInput").ap()
    out_d = nc.dram_tensor("out", [BL, R, H * D], FP32,
                           kind="ExternalOutput").ap()
    # DRAM bounce for dot-score transposition, one tensor per head-quad
    # per batch-pair (b0/b1 critical-path, b2/b3 deferred)
    scr01s = [nc.dram_tensor(f"scr01_{q}", [128, 4 * 2 * C], FP16,
                             kind="Internal").ap() for q in range(4)]
    scr23s = [nc.dram_tensor(f"scr23_{q}", [128, 4 * 2 * C], FP16,
                             kind="Internal").ap() for q in range(4)]

    with tile.TileContext(nc) as tc, ExitStack() as ctx:
        const_p = ctx.enter_context(tc.tile_pool(name="const", bufs=1))
        big_p = ctx.enter_context(tc.tile_pool(name="big", bufs=1))
        rhs_p = ctx.enter_context(tc.tile_pool(name="rhs", bufs=2))
        rr_p = ctx.enter_context(tc.tile_pool(name="rr", bufs=8))
        wsb_p = ctx.enter_context(tc.tile_pool(name="wsb", bufs=4))
        fout_p = ctx.enter_context(tc.tile_pool(name="fout", bufs=1))
        small_p = ctx.enter_context(tc.tile_pool(name="small", bufs=4))
        psA = ctx.enter_context(
            tc.tile_pool(name="psA", bufs=3, space="PSUM"))   # [128,1024] x3
        ps2_p = ctx.enter_context(
            tc.tile_pool(name="ps2", bufs=2, space="PSUM"))   # [128,512] x2

        bal = Balancer(nc)

        # ---- PE warmup: ~4us of dummy matmuls on a zeroed tile so the
        # p-state ramp (2-4x slower first 3us) completes before real work
        zwarm = const_p.tile([128, 512], FP16)
        nc.gpsimd.memset(zwarm[:], 0.0)
        wps = ps2_p.tile([128, 512], FP32, tag="ps2", name="wps")
        for i in range(7):
            nc.tensor.matmul(wps[:], zwarm[:, 0:128], zwarm[:],
                             start=(i == 0), stop=(i == 6))

        # ---- const loads (single DMAs, host-prepped layouts)
        xr2 = const_p.tile([128, 2 * TOK], FP16)   # [p, (eh, tok)]
        xc2 = const_p.tile([128, 2 * TOK], FP16)
        wqk16 = const_p.tile([128, 2 * 2 * 768], FP16)  # [p, (eh, q768|k768)]
        wv16 = const_p.tile([128, 2 * E], FP16)         # [p, (eh, 256)]
        # load order: q-proj needs only the q-halves of wqk + xr
        nc.sync.dma_start(wqk16[:, 0:768], wqk_d[:, 0:768])
        nc.sync.dma_start(xr2[:], xr_d[:])
        nc.sync.dma_start(wqk16[:, 1536:2304], wqk_d[:, 1536:2304])
        nc.sync.dma_start(xc2[:], xc_d[:])
        nc.sync.dma_start(wqk16[:, 768:1536], wqk_d[:, 768:1536])
        nc.sync.dma_start(wqk16[:, 2304:3072], wqk_d[:, 2304:3072])
        w1sb = const_p.tile([17, 256], FP16)
        nc.gpsimd.dma_start(w1sb[:], w1_d[:])
        w2sb = const_p.tile([128, 16], FP16)
        nc.gpsimd.dma_start(w2sb[:], w2_d[:])
        bc2 = const_p.tile([128, 2], FP32)
        nc.gpsimd.dma_start(bc2[:], bc_d[:])
        nc.gpsimd.dma_start(wv16[:], wv_d[:])

        # ---- prep, mh-major: proj(q,k) for mh then dots for its 4 heads,
        #      so scratch DMAs start as early as possible
        quads = {}
        x4all = big_p.tile([128, H * BL * C], FP16, name="x4all")

        def headT(proj, h):
            q6, sl = h // 3, h % 3
            t = quads[(proj, q6 // 2)]
            co = (q6 % 2) * TOK
            return t[sl * 32:sl * 32 + 16, co:co + TOK]

        # rhs tiles for b0/b1 up front; cost rows load immediately, dot
        # rows arrive as quad-sized stage2 DMAs pipelined into the mh loop
        # views [r, h4, b2, c] for the stage2 per-batch slices
        s01v = [sd[:].rearrange("r (h b c) -> r h b c", h=4, b=2)
                for sd in scr01s]
        s23v = [sd[:].rearrange("r (h b c) -> r h b c", h=4, b=2)
                for sd in scr23s]
        rhs0 = rhs_p.tile([17, PTS], FP16, tag="rhs", name="rhs0")
        rhs1 = rhs_p.tile([17, PTS], FP16, tag="rhs", name="rhs1")
        nc.scalar.dma_start(rhs0[16:17, :], cost_d[0:1, :])
        nc.scalar.dma_start(rhs1[16:17, :], cost_d[1:2, :])

        def emit_proj(pt):
            # pairtile pt covers 6-quads 2pt, 2pt+1 (3 heads each)
            for proj, qofs in (("q", 0), ("k", 768)):
                ps = psA.tile([128, 1024], FP32, tag="psA", name="ps")
                for qp in range(2):
                    q6 = pt * 2 + qp
                    for eh in range(2):
                        nc.tensor.matmul(
                            ps[0:96, qp * TOK:(qp + 1) * TOK],
                            wqk16[:, eh * 1536 + qofs + q6 * 128:
                                  eh * 1536 + qofs + q6 * 128 + 96],
                            xr2[:, eh * TOK:(eh + 1) * TOK] if proj == "q"
                            else xc2[:, eh * TOK:(eh + 1) * TOK],
                            start=(eh == 0), stop=(eh == 1))
                qt = big_p.tile([96, 2 * TOK], FP16, tag=f"{proj}T{pt}",
                                name=f"{proj}T{pt}")
                bal.copy(qt[:], ps[0:96, :], 1024)
                quads[(proj, pt)] = qt

        vhats = {}
        vh_tiles = {}
        for b in range(BL):
            vh = big_p.tile([128, 17 * H], FP16, tag=f"vhat{b}",
                            name=f"vhat{b}")
            vh_tiles[b] = vh
            nc.gpsimd.memset(
                vh[:].rearrange("p (h x) -> p h x", h=H)[:, :, 16:17], 1.0)

        def emit_vhat(b):
            vh = vh_tiles[b]
            vh3 = vh[:].rearrange("p (h x) -> p h x", h=H)
            ps = psA.tile([128, 1024], FP32, tag="psA", name="ps")
            for eh in range(2):
                nc.tensor.matmul(
                    ps[:, 0:E],
                    xc2[:, eh * TOK + b * 128:eh * TOK + (b + 1) * 128],
                    wv16[:, eh * E:(eh + 1) * E],
                    start=(eh == 0), stop=(eh == 1))
            bal.copy(vh3[:, :, 0:16],
                     ps[:, 0:E].rearrange("p (h x) -> p h x", h=H), E)
            vhats[b] = vh

        # proj runs 2 mh ahead of dots; vhat interleaved to fill gaps
        emit_proj(0)
        emit_proj(1)
        for mh in range(4):
            for hp2 in range(2):   # dots for heads 4mh .. 4mh+3
                ps = psA.tile([128, 1024], FP32, tag="psA", name="ps")
                for hh in range(2):
                    h = mh * 4 + hp2 * 2 + hh
                    for b in range(BL):
                        nc.tensor.matmul(
                            ps[:, hh * 512 + b * 128:hh * 512 + (b + 1) * 128],
                            headT("q", h)[:, b * 128:(b + 1) * 128],
                            headT("k", h)[:, b * 128:(b + 1) * 128])
                hbase = mh * 4 + hp2 * 2
                bal.copy(x4all[:, hbase * 512:(hbase + 2) * 512], ps[:], 1024)
            # stage1 for b0/b1 only (first 256 cols of each h-block)
            nc.sync.dma_start(
                scr01s[mh][:],
                x4all[:].rearrange("r (h x) -> r h x", h=H)
                [:, mh * 4:(mh + 1) * 4, 0:256])
            # stage2 quad DMA for b0 only (critical path); b1 deferred
            nc.sync.dma_start(
                rhs0[mh * 4:(mh + 1) * 4, :].rearrange(
                    "h (r c) -> h r c", c=C),
                s01v[mh][:, :, 0, :].rearrange("r h c -> h r c"))
            if mh == 1:
                emit_proj(2)

        for b in range(BL):
            emit_vhat(b)

        # ---- per-batch MLP + softmax + AV
        fouts = [fout_p.tile([128, H * D], FP32, tag=f"fo{b}", name=f"fo{b}")
                 for b in range(BL)]

        def assemble(b):
            rhs = rhs_p.tile([17, PTS], FP16, tag="rhs", name="rhs")
            # dots rows: dst (h; r, c) <- scratch [h, r, b, c] slice
            for q in range(4):
                nc.sync.dma_start(
                    rhs[q * 4:(q + 1) * 4, :].rearrange(
                        "h (r c) -> h r c", c=C),
                    s23v[q][:, :, b - 2, :].rearrange("r h c -> h r c"))
            nc.sync.dma_start(rhs[16:17, :], cost_d[b:b + 1, :])
            return rhs

        def make_tail(b, half, wsbs):
            # AV + normalize for (b, half), deferred into the next half's
            # pair loop so the PE never stalls on the ACT-queued exp
            def tail():
                psa = ps2_p.tile([128, 17 * 8], FP32, tag="ps2", name="psa")
                for grp in range(2):
                    w4 = wsbs[grp][:].rearrange("p (r h) -> p r h", h=8)
                    for hl in range(8):
                        h = half * 8 + hl
                        nc.tensor.matmul(
                            psa[grp * 64:(grp + 1) * 64,
                                hl * 17:hl * 17 + 17],
                            w4[:, :, hl],
                            vhats[b][:, h * 17:(h + 1) * 17])
                psa3 = psa[:].rearrange("p (x y) -> p x y", x=8)
                rec = small_p.tile([128, 8], FP32, tag="rec")
                nc.vector.reciprocal(rec[:], psa3[:, :, 16])
                bal.charge("D", 8)
                recb = rec[:].rearrange(
                    "p (h o) -> p h o", o=1).broadcast_to([128, 8, 16])
                nc.vector.tensor_tensor(
                    fouts[b][:, half * 128:(half + 1) * 128].rearrange(
                        "p (h x) -> p h x", h=8),
                    psa3[:, :, 0:16], recb, ALU.mult)
                bal.charge("D", 128)
                if half == 1:
                    nc.sync.dma_start(out_d[b], fouts[b][:])
            return tail

        # deferred: b1's stage2 quads and b2/b3 stage1 — emitted after
        # b0's full chain so their transfers never contend with it
        for q in range(4):
            nc.sync.dma_start(
                rhs1[q * 4:(q + 1) * 4, :].rearrange(
                    "h (r c) -> h r c", c=C),
                s01v[q][:, :, 1, :].rearrange("r h c -> h r c"))
        for q in range(4):
            nc.sync.dma_start(
                scr23s[q][:],
                x4all[:].rearrange("r (h x) -> r h x", h=H)
                [:, q * 4:(q + 1) * 4, 256:512])

        rhss = {0: rhs0, 1: rhs1}
        prev_tail = None
        for b in range(BL):
            if b + 2 < BL:
                rhss[b + 2] = assemble(b + 2)
            rhs = rhss.pop(b)
            for half in range(2):
                w1h = w1sb[:, half * 128:(half + 1) * 128]
                w2h = w2sb[:, half * 8:(half + 1) * 8]
                bch = bc2[:, half:half + 1]
                ps2s = []
                pend = []   # (rr, pair) with layer2 not yet emitted

                def emit_l2(rr, pair, ps2s=ps2s, w2h=w2h):
                    if pair % 8 == 0:
                        ps2t = ps2_p.tile([128, 512], FP32, tag="ps2",
                                          name="ps2t")
                        ps2s.append(ps2t)
                    for s in range(8):
                        rloc = (pair % 8) * 8 + s
                        nc.tensor.matmul(
                            ps2s[-1][:, rloc * 8:rloc * 8 + 8],
                            rr[:, s * 128:(s + 1) * 128], w2h)

                wsbs = []
                for pair in range(16):
                    ps1 = psA.tile([128, 1024], FP32, tag="psA")
                    for k in range(2):
                        ck = pair * 2 + k
                        nc.tensor.matmul(
                            ps1[:, k * 512:(k + 1) * 512], w1h,
                            rhs[:, ck * 512:(ck + 1) * 512])
                    rr = rr_p.tile([128, 1024], FP16, tag="rr")
                    if pair % 2 == 0:
                        nc.vector.tensor_scalar(rr[:], ps1[:], bch, 0.0,
                                                ALU.add, ALU.max)
                        bal.charge("D", 1024)
                    else:
                        nc.scalar.activation(rr[:], ps1[:], AF.Relu, bias=bch)
                        bal.charge("A", 1024)
                    pend.append((rr, pair))
                    if len(pend) > 4:
                        emit_l2(*pend.pop(0))
                    if pair == 4 and prev_tail is not None:
                        prev_tail()
                        prev_tail = None
                    # grp0 fully written once l2(p7) emitted -> exp it early
                    if pend and pend[0][1] == 8 and len(ps2s) == 1:
                        wsb = wsb_p.tile([128, 512], FP16, tag="wsb")
                        nc.scalar.activation(wsb[:], ps2s[0][:], AF.Exp)
                        bal.charge("A", 512)
                        wsbs.append(wsb)
                while pend:
                    emit_l2(*pend.pop(0))
                wsb = wsb_p.tile([128, 512], FP16, tag="wsb")
                nc.scalar.activation(wsb[:], ps2s[1][:], AF.Exp)
                bal.charge("A", 512)
                wsbs.append(wsb)
                prev_tail = make_tail(b, half, wsbs)

        prev_tail()

    nc.compile()
    return nc


_cache = {}


def _prep(inputs):
    row_emb = np.asarray(inputs["row_emb"], dtype=np.float32)
    col_emb = np.asarray(inputs["col_emb"], dtype=np.float32)
    cost_mat = np.asarray(inputs["cost_mat"], dtype=np.float32)
    Wq = np.asarray(inputs["Wq"], dtype=np.float32) / np.sqrt(D)
    Wk = np.asarray(inputs["Wk"], dtype=np.float32)
    Wv = np.asarray(inputs["Wv"], dtype=np.float32)
    m1w = np.asarray(inputs["mix1_weight"], dtype=np.float32)
    m1b = np.asarray(inputs["mix1_bias"], dtype=np.float32)
    m2w = np.asarray(inputs["mix2_weight"], dtype=np.float32)

    a1, c1 = m1w[:, 0, :], m1w[:, 1, :]
    w2 = m2w[:, :, 0]

    # layer1 stationary [17, 256]: col (half*128 + hl*16 + m)
    w1l = np.zeros((17, 256), dtype=np.float16)
    w2l = np.zeros((128, 16), dtype=np.float16)
    bcol2 = np.zeros((128, 2), dtype=np.float32)
    for h in range(H):
        half, hl = h // 8, h % 8
        for m in range(MS):
            col = half * 128 + hl * 16 + m
            w1l[h, col] = a1[h, m]
            w1l[16, col] = c1[h, m]
            w2l[hl * 16 + m, half * 8 + hl] = w2[h, m]
            bcol2[hl * 16 + m, half] = m1b[h, m]

    # padded q/k weights: head h -> 32-col slot (h%4)*32 within quad h//4
    def pad_qk(w):
        wp = np.zeros((E, 768), dtype=np.float16)
        w4 = w.reshape(E, H, D)
        for h in range(H):
            q6, sl = h // 3, h % 3
            wp[:, q6 * 128 + sl * 32:q6 * 128 + sl * 32 + 16] = w4[:, h, :]
        return wp

    def fold_eh(a):
        # [E, X] -> [128, (eh, X)]: row eh*128+p -> partition p, col-block eh
        x = a.reshape(2, 128, a.shape[1]).transpose(1, 0, 2)
        return np.ascontiguousarray(x.reshape(128, -1).astype(np.float16))

    wqk = fold_eh(np.concatenate([pad_qk(Wq), pad_qk(Wk)], axis=1))
    wv = fold_eh(Wv)

    per_core = []
    for i in range(NCORES):
        sl = slice(i * BL, (i + 1) * BL)
        xr = row_emb[sl].reshape(TOK, E).T
        xc = col_emb[sl].reshape(TOK, E).T
        per_core.append({
            "xrT": fold_eh(xr),
            "xcT": fold_eh(xc),
            "wqk": wqk,
            "wv": wv,
            "w1l": w1l,
            "w2l": w2l,
            "bcol2": bcol2,
            "cost16": cost_mat[sl].reshape(BL, PTS).astype(np.float16),
        })
    return per_core


def kernel(**inputs):
    if "nc" not in _cache:
        _cache["nc"] = build_kernel()
    nc = _cache["nc"]
    in_maps = _prep(inputs)
    res = run_bass_kernel_spmd(nc, in_maps, list(range(NCORES)))
    out = np.concatenate([res.results[i]["out"] for i in range(NCORES)],
                         axis=0)
    return out.astype(np.float32)
